# revision 5
# baseline (speedup 1.0000x reference)
"""Trainium2 Bass kernel for Conv2d(128->256, 3x3, stride 1, pad 1) on
x(32,128,56,56) fp32, data-parallel over batch across 8 NeuronCores.

Mapping: contraction dim = C_in=128 (SBUF partitions). Per (image, co-block)
the 7 row-groups (8 rows x 56 cols = 448 px) accumulate in 7 PSUM banks with
the 9 kernel taps as the OUTER loop, so 7 consecutive matmuls share one
stationary weight tile (walrus ldw-opt then skips 6 of 7 weight reloads).
Input is zero-padded to 58x58 on the host so every tap is a pure strided
window. The kernel is DMA-roofline bound, so x/w stream as bf16 and the
output is stored as bf16 and upcast to fp32 on the host (absmax rel err
~1e-3, well inside the 2e-2 gate).
"""
import numpy as np

N_CORES = 8
N_PER_CORE = 4          # 32 images / 8 cores
C_IN, C_OUT, K = 128, 256, 3
H = W = 56
HP = WP = 58            # padded
ROWS_PER_GROUP = 8
N_GROUPS = H // ROWS_PER_GROUP  # 7
NFREE = ROWS_PER_GROUP * W      # 448

# dtype mode: (x_dt, w_dt, o_dt) keys into mybir dtypes
MODE = ("bf16", "bf16", "bf16")

_compiled = {}


def _build(reps: int | None = None, mode=MODE):
    import concourse.bass as bass  # noqa: F401  (engine classes registered)
    import concourse.mybir as mybir
    import concourse.tile as tile
    from concourse import bacc

    # NOTE: bf16 matmuls lower to explicit InstLdweights+InstMatmult pairs,
    # which walrus's --enable-ldw-opt=true rejects — leave the flag at its
    # default (false). FWL + the PE reorder window hide bf16 weight loads.

    f32 = mybir.dt.float32
    dts = {"f32": f32, "f32r": mybir.dt.float32r, "bf16": mybir.dt.bfloat16}
    x_dt, w_dt, o_dt = (dts[m] for m in mode)

    nc = bacc.Bacc("TRN2", target_bir_lowering=False, debug=False,
                   num_devices=N_CORES)
    x_d = nc.declare_dram_parameter("x", [N_PER_CORE, C_IN, HP * WP], x_dt,
                                    isOutput=False)
    w_d = nc.declare_dram_parameter("w", [C_IN, K * K * C_OUT], w_dt,
                                    isOutput=False)
    b_d = nc.declare_dram_parameter("b", [128, 2], f32, isOutput=False)
    o_d = nc.declare_dram_parameter("o", [N_PER_CORE, 2, 128, H * W], o_dt,
                                    isOutput=True)

    with tile.TileContext(nc) as tc:
        with (
            tc.tile_pool(name="const", bufs=1) as const_pool,
            tc.tile_pool(name="xp", bufs=2) as x_pool,
            tc.tile_pool(name="op", bufs=2) as o_pool,
            tc.tile_pool(name="ps", bufs=8, space="PSUM") as psum_pool,
        ):
            b_sb = const_pool.tile([128, 2], f32, tag="b")
            w_sb = const_pool.tile([C_IN, K * K * C_OUT], w_dt, tag="w")
            x_first = x_pool.tile([C_IN, HP * WP], x_dt, tag="x")
            # weights + first image load; w first (first matmul needs tap 0)
            nc.sync.dma_start(w_sb[:, 0:256], w_d[:, 0:256])
            nc.sync.dma_start(x_first[:, 0:29 * WP], x_d[0, :, 0:29 * WP])
            nc.sync.dma_start(b_sb[:], b_d[:])
            nc.sync.dma_start(w_sb[:, 256:], w_d[:, 256:])
            nc.sync.dma_start(x_first[:, 29 * WP:], x_d[0, :, 29 * WP:])

            def body():
                for n in range(N_PER_CORE):
                    if n == 0:
                        x_sb = x_first
                    else:
                        x_sb = x_pool.tile([C_IN, HP * WP], x_dt, tag="x")
                        nc.sync.dma_start(x_sb[:], x_d[n])
                    x3 = x_sb[:].rearrange("p (h w) -> p h w", w=WP)
                    for cob in range(2):
                        o_sb = o_pool.tile([128, H * W], o_dt, tag="o")
                        pss = [psum_pool.tile([128, NFREE], f32, tag="ps",
                                              name=f"ps{rg}")
                               for rg in range(N_GROUPS)]
                        idx = 0
                        for kh in range(K):
                            for kw in range(K):
                                c0 = ((kh * K + kw) * 2 + cob) * 128
                                lhsT = w_sb[:, c0:c0 + 128]
                                for rg in range(N_GROUPS):
                                    r0 = rg * ROWS_PER_GROUP + kh
                                    rhs = x3[:, r0:r0 + ROWS_PER_GROUP,
                                             kw:kw + W]
                                    nc.tensor.matmul(
                                        pss[rg][:], lhsT, rhs,
                                        start=(idx == 0),
                                        stop=(idx == K * K - 1),
                                    )
                                idx += 1
                        for rg in range(N_GROUPS):
                            nc.vector.tensor_scalar_add(
                                o_sb[:, rg * NFREE:(rg + 1) * NFREE],
                                pss[rg][:], b_sb[:, cob:cob + 1],
                            )
                        # two half-block stores on the ACT hwdge ring keep
                        # issuance off the SP (load) ring and start draining
                        # before the whole block is done
                        half = (N_GROUPS // 2) * NFREE  # 1344
                        nc.scalar.dma_start(
                            o_d[n, cob][:, 0:half], o_sb[:, 0:half])
                        nc.scalar.dma_start(
                            o_d[n, cob][:, half:], o_sb[:, half:])

            if reps is None:
                body()
            else:
                with tc.For_i(0, reps, 1):
                    body()

    nc.compile()
    return nc


def _np_dt(mode_key):
    if mode_key == "bf16":
        import ml_dtypes
        return ml_dtypes.bfloat16
    return np.float32


def _prep_inputs(x, weight, bias, mode=MODE):
    """Host-side layout prep -> per-core in_maps."""
    x = np.asarray(x, dtype=np.float32)
    weight = np.asarray(weight, dtype=np.float32)
    bias = np.asarray(bias, dtype=np.float32)

    xp = np.pad(x, ((0, 0), (0, 0), (1, 1), (1, 1)))          # (32,128,58,58)
    xp = xp.reshape(N_CORES, N_PER_CORE, C_IN, HP * WP)
    xp = xp.astype(_np_dt(mode[0]))
    # weight (co, ci, kh, kw) -> (ci, kh, kw, cob, 128) flat [ci, 9*256]
    wr = weight.reshape(2, 128, C_IN, K, K).transpose(2, 3, 4, 0, 1)
    wr = np.ascontiguousarray(wr).reshape(C_IN, K * K * C_OUT)
    wr = wr.astype(_np_dt(mode[1]))
    br = np.ascontiguousarray(bias.reshape(2, 128).T)          # [128, 2]

    return [
        {"x": np.ascontiguousarray(xp[c]), "w": wr, "b": br}
        for c in range(N_CORES)
    ]


def kernel(x: np.ndarray, weight: np.ndarray, bias: np.ndarray) -> np.ndarray:
    from concourse.bass_utils import run_bass_kernel_spmd

    if "nc" not in _compiled:
        _compiled["nc"] = _build()
    nc = _compiled["nc"]

    in_maps = _prep_inputs(x, weight, bias)
    res = run_bass_kernel_spmd(nc, in_maps, list(range(N_CORES)))
    out = np.stack([np.asarray(r["o"], dtype=np.float32)
                    for r in res.results])                     # (8,4,2,128,3136)
    out = out.reshape(N_CORES * N_PER_CORE, C_OUT, H, W)
    return out


# revision 13
# speedup vs baseline: 1.4842x; 1.4842x over previous
"""Trainium2 Bass kernel for Conv2d(128->256, 3x3, stride 1, pad 1) on
x(32,128,56,56) fp32, data-parallel over batch across 8 NeuronCores.

Mapping: contraction dim = C_in=128 (SBUF partitions). Per (image, co-block)
the 7 row-groups (8 rows x 56 cols = 448 px) accumulate in 7 PSUM banks with
the 9 kernel taps as the OUTER loop, so 7 consecutive matmuls share one
stationary weight tile (walrus ldw-opt then skips 6 of 7 weight reloads).
Input is zero-padded to 58x58 on the host so every tap is a pure strided
window. The kernel is DMA-roofline bound, so x/w stream as bf16 and the
output is stored as bf16 and upcast to fp32 on the host (absmax rel err
~1e-3, well inside the 2e-2 gate).
"""
import numpy as np

N_CORES = 8
N_PER_CORE = 4          # 32 images / 8 cores
C_IN, C_OUT, K = 128, 256, 3
H = W = 56
HP = WP = 58            # padded
ROWS_PER_GROUP = 8
N_GROUPS = H // ROWS_PER_GROUP  # 7
NFREE = ROWS_PER_GROUP * W      # 448

# dtype mode: (x_dt, w_dt, o_dt) keys into mybir dtypes
MODE = ("f32r", "f32r", "bf16")

_compiled = {}


def _patch_ldw_opt():
    # walrus ships with --enable-ldw-opt=false hardcoded; enabling the
    # weight-load optimization lets self-loading f32r matmuls overlap the
    # stationary load with the previous matmul's stream. Only valid for
    # builds with no explicit InstLdweights (i.e. f32r moving operand).
    import concourse.bass_utils as bu

    if getattr(bu.run_command, "_ldw_patched", False):
        return
    orig = bu.run_command

    def patched(argv, **kw):
        argv = ["--enable-ldw-opt=true" if a == "--enable-ldw-opt=false" else a
                for a in argv]
        return orig(argv, **kw)

    patched._ldw_patched = True
    bu.run_command = patched


def _build(reps: int | None = None, mode=MODE, bias_split=True):
    import concourse.bass as bass  # noqa: F401  (engine classes registered)
    import concourse.mybir as mybir
    import concourse.tile as tile
    from concourse import bacc

    # NOTE: a bf16 MOVING operand lowers to explicit InstLdweights+InstMatmult
    # pairs, which --enable-ldw-opt=true rejects (and which serialize badly
    # with it off: measured ~280ns/MM). Keep the moving operand f32r so
    # matmuls self-load their stationary, and enable ldw-opt.
    assert mode[0] != "bf16", "bf16 moving operand is slow on this toolchain"
    _patch_ldw_opt()

    f32 = mybir.dt.float32
    dts = {"f32": f32, "f32r": mybir.dt.float32r, "bf16": mybir.dt.bfloat16}
    x_dt, w_dt, o_dt = (dts[m] for m in mode)

    nc = bacc.Bacc("TRN2", target_bir_lowering=False, debug=False,
                   num_devices=N_CORES)
    x_d = nc.declare_dram_parameter("x", [N_PER_CORE, C_IN, HP * WP], x_dt,
                                    isOutput=False)
    w_d = nc.declare_dram_parameter("w", [C_IN, K * K * C_OUT], w_dt,
                                    isOutput=False)
    b_d = nc.declare_dram_parameter("b", [128, 2], f32, isOutput=False)
    o_d = nc.declare_dram_parameter("o", [N_PER_CORE, 2, 128, H * W], o_dt,
                                    isOutput=True)

    with tile.TileContext(nc) as tc:
        with (
            tc.tile_pool(name="const", bufs=1) as const_pool,
            tc.tile_pool(name="xp", bufs=4) as x_pool,
            tc.tile_pool(name="op", bufs=2) as o_pool,
            tc.tile_pool(name="ps", bufs=8, space="PSUM") as psum_pool,
        ):
            b_sb = const_pool.tile([128, 2], f32, tag="b")
            w_sb = const_pool.tile([C_IN, K * K * C_OUT], w_dt, tag="w")
            x_first = x_pool.tile([C_IN, HP * WP], x_dt, tag="x")
            # weights + first image load; w first (first matmul needs tap 0)
            nc.sync.dma_start(w_sb[:, 0:256], w_d[:, 0:256])
            nc.sync.dma_start(x_first[:, 0:29 * WP], x_d[0, :, 0:29 * WP])
            nc.sync.dma_start(b_sb[:], b_d[:])
            nc.sync.dma_start(w_sb[:, 256:], w_d[:, 256:])
            nc.sync.dma_start(x_first[:, 29 * WP:], x_d[0, :, 29 * WP:])

            def body():
                # issue every image load upfront so the DMA pipe is packed
                # from t=0 (loads on the SP ring; stores go on ACT's)
                x_tiles = [x_first]
                for n in range(1, N_PER_CORE):
                    x_sb = x_pool.tile([C_IN, HP * WP], x_dt, tag="x",
                                       name=f"x{n}")
                    nc.sync.dma_start(x_sb[:], x_d[n])
                    x_tiles.append(x_sb)
                for n in range(N_PER_CORE):
                    x3 = x_tiles[n][:].rearrange("p (h w) -> p h w", w=WP)
                    for cob in range(2):
                        o_sb = o_pool.tile([128, H * W], o_dt, tag="o")
                        for rg in range(N_GROUPS):
                            # 9 taps accumulate back-to-back into ONE psum
                            # bank — consecutive same-bank matmuls pipeline
                            # at full rate (bank-cycling between MMs stalls
                            # the PE; measured 3.8x slower)
                            ps = psum_pool.tile([128, NFREE], f32, tag="ps")
                            idx = 0
                            for kh in range(K):
                                for kw in range(K):
                                    r0 = rg * ROWS_PER_GROUP + kh
                                    rhs = x3[:, r0:r0 + ROWS_PER_GROUP,
                                             kw:kw + W]
                                    c0 = ((kh * K + kw) * 2 + cob) * 128
                                    lhsT = w_sb[:, c0:c0 + 128]
                                    nc.tensor.matmul(
                                        ps[:], lhsT, rhs,
                                        start=(idx == 0),
                                        stop=(idx == K * K - 1),
                                    )
                                    idx += 1
                            dst = o_sb[:, rg * NFREE:(rg + 1) * NFREE]
                            if bias_split and rg % 2 == 1:
                                # odd row-groups on the (otherwise idle) ACT
                                # engine so PSUM->SBUF conversion doesn't
                                # bottleneck on DVE
                                nc.scalar.activation(
                                    dst, ps[:],
                                    mybir.ActivationFunctionType.Identity,
                                    bias=b_sb[:, cob:cob + 1], scale=1.0,
                                )
                            else:
                                nc.vector.tensor_scalar_add(
                                    dst, ps[:], b_sb[:, cob:cob + 1],
                                )
                        # two half-block stores on the ACT ring: keeps the SP
                        # (load) queue free of stores whose data isn't ready
                        # yet (engine dma_starts stall on their waits)
                        half = (N_GROUPS // 2) * NFREE  # 1344
                        nc.scalar.dma_start(
                            o_d[n, cob][:, 0:half], o_sb[:, 0:half])
                        nc.scalar.dma_start(
                            o_d[n, cob][:, half:], o_sb[:, half:])

            if reps is None:
                body()
            else:
                with tc.For_i(0, reps, 1):
                    body()

    nc.compile()
    return nc


def _np_dt(mode_key):
    if mode_key == "bf16":
        import ml_dtypes
        return ml_dtypes.bfloat16
    return np.float32


def _prep_inputs(x, weight, bias, mode=MODE):
    """Host-side layout prep -> per-core in_maps."""
    x = np.asarray(x, dtype=np.float32)
    weight = np.asarray(weight, dtype=np.float32)
    bias = np.asarray(bias, dtype=np.float32)

    xp = np.pad(x, ((0, 0), (0, 0), (1, 1), (1, 1)))          # (32,128,58,58)
    xp = xp.reshape(N_CORES, N_PER_CORE, C_IN, HP * WP)
    xp = xp.astype(_np_dt(mode[0]))
    # weight (co, ci, kh, kw) -> (ci, kh, kw, cob, 128) flat [ci, 9*256]
    wr = weight.reshape(2, 128, C_IN, K, K).transpose(2, 3, 4, 0, 1)
    wr = np.ascontiguousarray(wr).reshape(C_IN, K * K * C_OUT)
    wr = wr.astype(_np_dt(mode[1]))
    br = np.ascontiguousarray(bias.reshape(2, 128).T)          # [128, 2]

    return [
        {"x": np.ascontiguousarray(xp[c]), "w": wr, "b": br}
        for c in range(N_CORES)
    ]


def kernel(x: np.ndarray, weight: np.ndarray, bias: np.ndarray) -> np.ndarray:
    from concourse.bass_utils import run_bass_kernel_spmd

    if "nc" not in _compiled:
        _compiled["nc"] = _build()
    nc = _compiled["nc"]

    in_maps = _prep_inputs(x, weight, bias)
    res = run_bass_kernel_spmd(nc, in_maps, list(range(N_CORES)))
    out = np.stack([np.asarray(r["o"], dtype=np.float32)
                    for r in res.results])                     # (8,4,2,128,3136)
    out = out.reshape(N_CORES * N_PER_CORE, C_OUT, H, W)
    return out
